# revision 17
# baseline (speedup 1.0000x reference)
"""Trainium2 Bass kernel for ContextGatedTokenizer (GQA cross-attention tokenizer).

Computes, per batch b:
    q  = rmsnorm(target @ wq^T) ; k = rmsnorm((history @ wk^T).reshape(S,KVH,D))
    scores = (q grouped) @ k^T * D^-0.5 + mask_bias ; attn = softmax(scores)
    out = attn @ (history @ wv^T) ; returns (out, attn_weights)

Sharding: data-parallel over batch, 4 batches per core on 8 NeuronCores.

Kernel algebra (per core, per batch):
  - v-projection eliminated:  out = ((exp @ history) @ wv^T) * recip_denom
  - q-side folded into a tiny block-diagonal matrix Q~ [512,8] carrying
    q-rmsnorm, (1+gq)(1+gk) and D^-0.5; scores_raw = (wk^T Q~)^T @ history^T
  - k is computed only to extract per-(s,h) rms-norm factors:
    square (ACT) + ones-matmul partition-reduce (PE)
  - softmax without max-subtraction (|scores| <= ~16 by Cauchy-Schwarz after
    both rms-norms), normalization deferred to after the context matmul.
  - matmul operands in float32r (4x faster PE streaming, ~1e-4 matmul rel err)
  - rms factors via ACT Abs_reciprocal_sqrt (shares ACT table with Square);
    Exp ops batched per-batch to minimize ACT table reloads.
"""

import sys
from contextlib import ExitStack

if "/opt/trn_rl_repo" not in sys.path:
    sys.path.insert(0, "/opt/trn_rl_repo")

import numpy as np

import concourse.bass as bass
import concourse.bacc as bacc
import concourse.tile as tile
import concourse.mybir as mybir
from concourse import bass_utils
from concourse.masks import make_identity

F32 = mybir.dt.float32
F32R = mybir.dt.float32r
U8 = mybir.dt.uint8
AF = mybir.ActivationFunctionType
ALU = mybir.AluOpType

# Problem dims (hardcoded per spec)
B_FULL, K, S, D, KVH = 32, 8, 2048, 256, 2
NCORES = 8
BL = B_FULL // NCORES          # 4 batches per core
F = KVH * D                    # 512 = k/v projection width
G = K // KVH                   # 4 query heads per kv head
EPS = 1e-6
SCALING = D ** -0.5
NST = 4                        # s-tiles per batch (512 positions each)
STW = S // NST                 # 512
NSC = S // 128                 # 16 s-chunks of 128
MASK_NEG = -1.0e30


def build_kernel(nc, tc, ctx):
    tgt_d = nc.dram_tensor("target", [BL, K, D], F32R, kind="ExternalInput")
    hist_d = nc.dram_tensor("history", [BL, S, D], F32R, kind="ExternalInput")
    mask_d = nc.dram_tensor("mask", [BL, S], U8, kind="ExternalInput")
    wq_d = nc.dram_tensor("wq", [D, D], F32R, kind="ExternalInput")
    wk_d = nc.dram_tensor("wk", [F, D], F32R, kind="ExternalInput")
    wv_d = nc.dram_tensor("wv", [F, D], F32R, kind="ExternalInput")
    gq_d = nc.dram_tensor("gq", [1, D], F32, kind="ExternalInput")
    gk_d = nc.dram_tensor("gk", [1, D], F32, kind="ExternalInput")
    out_d = nc.dram_tensor("out", [BL, K, D], F32, kind="ExternalOutput")
    attn_d = nc.dram_tensor("attnw", [BL, K, S], F32, kind="ExternalOutput")

    consts = ctx.enter_context(tc.tile_pool(name="consts", bufs=1))
    # Single PSUM pool; per-tag bufs sum to exactly 8 banks:
    # ptr:2 + pk:2 + psmall:4
    psum = ctx.enter_context(tc.tile_pool(name="psum", bufs=1, space="PSUM"))
    hpool = ctx.enter_context(tc.tile_pool(name="hpool", bufs=10))
    htpool = ctx.enter_context(tc.tile_pool(name="htpool", bufs=4))
    sqpool = ctx.enter_context(tc.tile_pool(name="sqpool", bufs=6))
    bpool = ctx.enter_context(tc.tile_pool(name="bpool", bufs=3))
    stpool = ctx.enter_context(tc.tile_pool(name="stpool", bufs=3))

    def ptile(tag, bufs, shape=(128, STW), dtype=F32):
        return psum.tile(list(shape), dtype, tag=tag, bufs=bufs, name=tag)

    identf = consts.tile([128, 128], F32)
    make_identity(nc, identf)
    ident = consts.tile([128, 128], F32R)
    nc.vector.tensor_copy(ident, identf)
    epsb = consts.tile([128, 1], F32)
    nc.vector.memset(epsb, EPS)

    # ---- load weights (natural layouts, f32r) ----
    wk_nat = consts.tile([128, 4, D], F32R)    # [f%128, f//128, e]
    nc.sync.dma_start(out=wk_nat, in_=wk_d.rearrange("(c p) e -> p c e", p=128))
    wv_nat = consts.tile([128, 4, D], F32R)
    nc.sync.dma_start(out=wv_nat, in_=wv_d.rearrange("(c p) e -> p c e", p=128))
    wq_nat = consts.tile([128, 2, D], F32R)    # [e%128, e//128, d]
    nc.sync.dma_start(out=wq_nat, in_=wq_d.rearrange("(c p) e -> p c e", p=128))

    # ---- transposed weights: wkT/wvT [e, f] as [128, ec, 512]; wqT [d, e] ----
    wkT = consts.tile([128, 2, F], F32R)
    wvT = consts.tile([128, 2, F], F32R)
    for (w_nat, wT) in ((wk_nat, wkT), (wv_nat, wvT)):
        for ec in range(2):
            ps = ptile("ptr", 2, dtype=F32R)
            for fc in range(4):
                nc.tensor.transpose(
                    ps[:, fc * 128:(fc + 1) * 128],
                    w_nat[:, fc, ec * 128:(ec + 1) * 128],
                    ident,
                )
            nc.vector.tensor_copy(wT[:, ec, :], ps)
    wqT = consts.tile([128, 2, D], F32R)       # [d%128, d//128, e]
    for dc in range(2):
        ps = ptile("ptr", 2, dtype=F32R)
        for ec in range(2):
            nc.tensor.transpose(
                ps[:, ec * 128:(ec + 1) * 128],
                wq_nat[:, ec, dc * 128:(dc + 1) * 128],
                ident,
            )
        nc.vector.tensor_copy(wqT[:, dc, :], ps[:, 0:D])

    # ---- gamma product vector: gprodT[d%128, d//128] = (1+gq)*(1+gk)*SCALING ----
    gqT_t = consts.tile([128, 2], F32)
    gkT_t = consts.tile([128, 2], F32)
    nc.sync.dma_start(out=gqT_t, in_=gq_d.rearrange("a (c p) -> (a p) c", p=128))
    nc.sync.dma_start(out=gkT_t, in_=gk_d.rearrange("a (c p) -> (a p) c", p=128))
    gq1 = consts.tile([128, 2], F32)
    gk1 = consts.tile([128, 2], F32)
    nc.vector.tensor_scalar_add(gq1, gqT_t, 1.0)
    nc.vector.tensor_scalar_add(gk1, gkT_t, 1.0)
    gboth = consts.tile([128, 2], F32)
    nc.vector.tensor_tensor(out=gboth, in0=gq1, in1=gk1, op=ALU.mult)
    gprodT = consts.tile([128, 2], F32)       # [d%128, d//128]
    nc.vector.tensor_scalar_mul(gprodT, gboth, SCALING)

    # ---- ones patterns for the sumsq partition-reduce matmuls ----
    onesPf = consts.tile([128, 2, K], F32)
    nc.vector.memset(onesPf, 0.0)
    nc.vector.memset(onesPf[:, 0, 0:G], 1.0)
    nc.vector.memset(onesPf[:, 1, G:K], 1.0)
    onesP = consts.tile([128, 2, K], F32R)
    nc.vector.tensor_copy(onesP, onesPf)

    # ---- mask, broadcast to 8 partitions: [8, b*st, 512] (uint8) ----
    mask_rep = consts.tile([K, BL * NST, STW], U8)
    mask_ap = mask_d[:, :]
    mask_bcast = bass.AP(
        tensor=mask_ap.tensor,
        offset=mask_ap.offset,
        ap=[[0, K], [STW, BL * NST], [1, STW]],
    )
    nc.gpsimd.dma_start(out=mask_rep, in_=mask_bcast)

    # ================= per-batch pipeline =================
    for b in range(BL):
        # ---- q path ----
        tgt = bpool.tile([K, D], F32R, tag="tgt", name="tgt")
        nc.sync.dma_start(out=tgt, in_=tgt_d[b])
        ps_t = ptile("ptr", 2, dtype=F32R)
        for dc in range(2):
            nc.tensor.transpose(
                ps_t[:, dc * K:(dc + 1) * K],
                tgt[:, dc * 128:(dc + 1) * 128],
                ident[0:K, 0:K],
            )
        tgtT = bpool.tile([128, 2 * K], F32R, tag="tgtT", name="tgtT")
        nc.vector.tensor_copy(tgtT, ps_t[:, 0:2 * K])

        ps_q = ptile("psmall", 4, (K, F))
        for dc in range(2):
            nc.tensor.matmul(
                ps_q[:, 0:D], tgtT[:, dc * K:(dc + 1) * K], wqT[:, dc, :],
                start=(dc == 0), stop=(dc == 1),
            )
        qsq = bpool.tile([K, D], F32, tag="qsq", name="qsq")
        ssq = bpool.tile([K, 1], F32, tag="ssq", name="ssq")
        nc.scalar.activation(qsq, ps_q[:, 0:D], AF.Square, accum_out=ssq)
        lnq = bpool.tile([K, 1], F32, tag="lnq", name="lnq")
        nc.scalar.activation(lnq, ssq, AF.Ln, bias=epsb[0:K], scale=1.0 / D)
        rq = bpool.tile([K, 1], F32, tag="rq", name="rq")
        nc.scalar.activation(rq, lnq, AF.Exp, scale=-0.5)
        qn = bpool.tile([K, D], F32R, tag="qn", name="qn")
        nc.vector.tensor_scalar_mul(qn, ps_q[:, 0:D], rq)

        ps_qn = ptile("ptr", 2, dtype=F32R)
        for dc in range(2):
            nc.tensor.transpose(
                ps_qn[:, dc * K:(dc + 1) * K],
                qn[:, dc * 128:(dc + 1) * 128],
                ident[0:K, 0:K],
            )
        qnT = bpool.tile([128, 2 * K], F32R, tag="qnT", name="qnT")
        nc.vector.tensor_copy(qnT, ps_qn[:, 0:2 * K])

        Qtf = bpool.tile([128, 4, K], F32, tag="Qtf", name="Qtf")
        nc.vector.memset(Qtf, 0.0)
        for c in range(4):
            h, dc = c // 2, c % 2
            nc.vector.tensor_scalar_mul(
                Qtf[:, c, h * G:(h + 1) * G],
                qnT[:, dc * K + h * G: dc * K + (h + 1) * G],
                gprodT[:, dc:dc + 1],
            )
        Qt = bpool.tile([128, 4, K], F32R, tag="Qt", name="Qt")
        nc.vector.tensor_copy(Qt, Qtf)

        ps_pt = ptile("psmall", 4, (K, F))
        for c in range(4):
            nc.tensor.matmul(
                ps_pt[:, 0:D], Qt[:, c, :], wk_nat[:, c, :],
                start=(c == 0), stop=(c == 3),
            )
        PT_sb = bpool.tile([K, D], F32R, tag="PT_sb", name="PT_sb")
        nc.vector.tensor_copy(PT_sb, ps_pt[:, 0:D])
        ps_p = ptile("ptr", 2, dtype=F32R)
        for ec in range(2):
            nc.tensor.transpose(
                ps_p[:, ec * K:(ec + 1) * K],
                PT_sb[:, ec * 128:(ec + 1) * 128],
                ident[0:K, 0:K],
            )
        P_sb = bpool.tile([128, 2 * K], F32R, tag="P_sb", name="P_sb")
        nc.vector.tensor_copy(P_sb, ps_p[:, 0:2 * K])

        h_tiles = []
        score_tiles = []
        for st in range(NST):
            h_nat = hpool.tile([128, 4, D], F32R, tag="h_nat", name="h_nat")
            nc.sync.dma_start(
                out=h_nat,
                in_=hist_d[b].rearrange("(t p) e -> p t e", p=128)[
                    :, st * 4:(st + 1) * 4, :
                ],
            )
            h_tiles.append(h_nat)

            # transpose history tile: hT [e, s] chunks
            hT = htpool.tile([128, 2, STW], F32R, tag="hT", name="hT")
            for ec in range(2):
                ps_h = ptile("ptr", 2, dtype=F32R)
                for j2 in range(4):
                    nc.tensor.transpose(
                        ps_h[:, j2 * 128:(j2 + 1) * 128],
                        h_nat[:, j2, ec * 128:(ec + 1) * 128],
                        ident,
                    )
                nc.vector.tensor_copy(hT[:, ec, :], ps_h)

            # scores_raw [8, 512]
            ps_sc = ptile("psmall", 4, (K, STW))
            nc.tensor.matmul(ps_sc, P_sb[:, 0:K], hT[:, 0, :], start=True, stop=False)
            nc.tensor.matmul(ps_sc, P_sb[:, K:2 * K], hT[:, 1, :], start=False,
                             stop=True)

            # k-projection (kT layout) + squares + sumsq partition-reduce
            ps_ss = ptile("psmall", 4, (K, STW))
            sqs = []
            for c in range(4):
                ps_k = ptile("pk", 2)
                nc.tensor.matmul(
                    ps_k, wkT[:, 0, c * 128:(c + 1) * 128], hT[:, 0, :],
                    start=True, stop=False,
                )
                nc.tensor.matmul(
                    ps_k, wkT[:, 1, c * 128:(c + 1) * 128], hT[:, 1, :],
                    start=False, stop=True,
                )
                sq = sqpool.tile([128, STW], F32, tag="sq", name="sq")
                nc.scalar.activation(sq, ps_k, AF.Square)
                sqs.append(sq)
            for h in range(KVH):
                sqp = sqpool.tile([128, STW], F32R, tag="sqp", name="sqp")
                nc.gpsimd.tensor_tensor(
                    out=sqp, in0=sqs[2 * h], in1=sqs[2 * h + 1], op=ALU.add
                )
                nc.tensor.matmul(
                    ps_ss, onesP[:, h, :], sqp, start=(h == 0), stop=(h == 1)
                )

            # r = rsqrt(mean+eps) = exp(-0.5*ln(mean+eps)); single ACT table
            lnm = stpool.tile([K, STW], F32, tag="lnm", name="lnm")
            nc.scalar.activation(lnm, ps_ss, AF.Ln, bias=epsb[0:K], scale=1.0 / D)
            rrec = stpool.tile([K, STW], F32, tag="rrec", name="rrec")
            nc.scalar.activation(rrec, lnm, AF.Exp, scale=-0.5)
            scorest = stpool.tile([K, STW], F32, tag="scorest", name="scorest")
            nc.vector.tensor_mul(scorest, ps_sc, rrec)
            scores = stpool.tile([K, STW], F32, tag="scores", name="scores", bufs=6)
            nc.vector.scalar_tensor_tensor(
                out=scores, in0=mask_rep[:, b * NST + st, :], scalar=MASK_NEG,
                in1=scorest, op0=ALU.mult, op1=ALU.add,
            )
            score_tiles.append(scores)

        # ---- batched exp (one ACT table switch per batch) ----
        exp_tiles = []
        sum_tiles = []
        for st in range(NST):
            exp_st = stpool.tile([K, STW], F32R, tag="exp_st", name="exp_st", bufs=6)
            expsum = stpool.tile([K, 1], F32, tag="expsum", name="expsum", bufs=6)
            nc.scalar.activation(exp_st, score_tiles[st], AF.Exp, accum_out=expsum)
            exp_tiles.append(exp_st)
            sum_tiles.append(expsum)

        # ---- denominator + attn output ----
        dn01 = bpool.tile([K, 1], F32, tag="dn01", name="dn01")
        nc.vector.tensor_add(dn01, sum_tiles[0], sum_tiles[1])
        dn23 = bpool.tile([K, 1], F32, tag="dn23", name="dn23")
        nc.vector.tensor_add(dn23, sum_tiles[2], sum_tiles[3])
        denom = bpool.tile([K, 1], F32, tag="denom", name="denom")
        nc.vector.tensor_add(denom, dn01, dn23)
        recip = bpool.tile([K, 1], F32, tag="recip", name="recip")
        nc.vector.reciprocal(recip, denom)

        for st in range(NST):
            attn_st = stpool.tile([K, STW], F32, tag="attn_st", name="attn_st")
            nc.vector.tensor_scalar_mul(attn_st, exp_tiles[st], recip)
            nc.sync.dma_start(
                out=attn_d[b][:, st * STW:(st + 1) * STW], in_=attn_st
            )

        # ---- context: expT chunks then ctx = exp @ history ----
        ps_et = ptile("ptr", 2, dtype=F32R)
        for sc in range(NSC):
            st, cc = sc // 4, sc % 4
            nc.tensor.transpose(
                ps_et[:, sc * K:(sc + 1) * K],
                exp_tiles[st][:, cc * 128:(cc + 1) * 128],
                ident[0:K, 0:K],
            )
        expT = bpool.tile([128, NSC * K], F32R, tag="expT", name="expT")
        nc.vector.tensor_copy(expT, ps_et[:, 0:NSC * K])

        ps_ctx = ptile("psmall", 4, (K, F))
        for sc in range(NSC):
            nc.tensor.matmul(
                ps_ctx[:, 0:D], expT[:, sc * K:(sc + 1) * K],
                h_tiles[sc // 4][:, sc % 4, :],
                start=(sc == 0), stop=(sc == NSC - 1),
            )
        ctx_sb = bpool.tile([K, D], F32R, tag="ctx_sb", name="ctx_sb")
        nc.vector.tensor_copy(ctx_sb, ps_ctx[:, 0:D])
        ps_ct = ptile("ptr", 2, dtype=F32R)
        for ec in range(2):
            nc.tensor.transpose(
                ps_ct[:, ec * K:(ec + 1) * K],
                ctx_sb[:, ec * 128:(ec + 1) * 128],
                ident[0:K, 0:K],
            )
        ctxT = bpool.tile([128, 2 * K], F32R, tag="ctxT", name="ctxT")
        nc.vector.tensor_copy(ctxT, ps_ct[:, 0:2 * K])

        # ---- out = (ctx @ wv^T) * recip, via block-diagonal ctx_aug ----
        ctx_augf = bpool.tile([128, 4, K], F32, tag="ctx_augf", name="ctx_augf")
        nc.vector.memset(ctx_augf, 0.0)
        for c in range(4):
            h, ec = c // 2, c % 2
            nc.vector.tensor_copy(
                ctx_augf[:, c, h * G:(h + 1) * G],
                ctxT[:, ec * K + h * G: ec * K + (h + 1) * G],
            )
        ctx_aug = bpool.tile([128, 4, K], F32R, tag="ctx_aug", name="ctx_aug")
        nc.vector.tensor_copy(ctx_aug, ctx_augf)
        ps_o = ptile("psmall", 4, (K, F))
        for c in range(4):
            nc.tensor.matmul(
                ps_o[:, 0:D], ctx_aug[:, c, :],
                wvT[:, c % 2, (c // 2) * D:((c // 2) + 1) * D],
                start=(c == 0), stop=(c == 3),
            )
        out_sb = bpool.tile([K, D], F32, tag="out_sb", name="out_sb")
        nc.vector.tensor_scalar_mul(out_sb, ps_o[:, 0:D], recip)
        nc.sync.dma_start(out=out_d[b], in_=out_sb)


def _patch_act_tables():
    # Force all our ACT funcs (Square/Ln/Exp/Copy) onto the one shared table
    # so the table-load pass never alternates tables between them.
    import concourse.hw_specs as hw_specs
    if getattr(bacc, "_act_tables_patched", False):
        return
    orig = hw_specs.get_activation_tables

    def patched(arch):
        t = orig(arch)
        shared = t.get("natural_log_exp_and_others")
        if shared is None:
            return t
        return {
            name: (funcs if name == "natural_log_exp_and_others"
                   else funcs - shared)
            for name, funcs in t.items()
        }

    bacc.get_activation_tables = patched
    bacc._act_tables_patched = True


def build_program():
    _patch_act_tables()
    nc = bacc.Bacc("TRN2", target_bir_lowering=False)
    with tile.TileContext(nc) as tc:
        with ExitStack() as ctx:
            build_kernel(nc, tc, ctx)
    nc.compile()
    return nc


_program = None


def _get_program():
    global _program
    if _program is None:
        _program = build_program()
    return _program


def kernel(**inputs):
    target_emb = np.asarray(inputs["target_emb"], dtype=np.float32)
    history_emb = np.asarray(inputs["history_emb"], dtype=np.float32)
    mask = np.asarray(inputs["key_padding_mask"]).astype(np.uint8)
    wq = np.asarray(inputs["wq"], dtype=np.float32)
    wk = np.asarray(inputs["wk"], dtype=np.float32)
    wv = np.asarray(inputs["wv"], dtype=np.float32)
    gq = np.asarray(inputs["gq"], dtype=np.float32).reshape(1, D)
    gk = np.asarray(inputs["gk"], dtype=np.float32).reshape(1, D)

    nc = _get_program()
    in_maps = []
    for i in range(NCORES):
        sl = slice(i * BL, (i + 1) * BL)
        in_maps.append({
            "target": target_emb[sl],
            "history": history_emb[sl],
            "mask": mask[sl],
            "wq": wq, "wk": wk, "wv": wv, "gq": gq, "gk": gk,
        })
    res = bass_utils.run_bass_kernel_spmd(nc, in_maps, core_ids=list(range(NCORES)))
    out = np.concatenate([res.results[i]["out"] for i in range(NCORES)], axis=0)
    attn = np.concatenate([res.results[i]["attnw"] for i in range(NCORES)], axis=0)
    return out, attn.reshape(B_FULL, K, 1, S)


# revision 18
# speedup vs baseline: 1.0294x; 1.0294x over previous
"""Trainium2 Bass kernel for ContextGatedTokenizer (GQA cross-attention tokenizer).

Computes, per batch b:
    q  = rmsnorm(target @ wq^T) ; k = rmsnorm((history @ wk^T).reshape(S,KVH,D))
    scores = (q grouped) @ k^T * D^-0.5 + mask_bias ; attn = softmax(scores)
    out = attn @ (history @ wv^T) ; returns (out, attn_weights)

Sharding: data-parallel over batch, 4 batches per core on 8 NeuronCores.

Kernel algebra (per core, per batch):
  - v-projection eliminated:  out = ((exp @ history) @ wv^T) * recip_denom
  - q-side folded into a tiny block-diagonal matrix Q~ [512,8] carrying
    q-rmsnorm, (1+gq)(1+gk) and D^-0.5; scores_raw = (wk^T Q~)^T @ history^T
  - k is computed only to extract per-(s,h) rms-norm factors:
    square (ACT) + ones-matmul partition-reduce (PE)
  - softmax without max-subtraction (|scores| <= ~16 by Cauchy-Schwarz after
    both rms-norms), normalization deferred to after the context matmul.
  - matmul operands in float32r (4x faster PE streaming, ~1e-4 matmul rel err)
  - rms factors via ACT Abs_reciprocal_sqrt (shares ACT table with Square);
    Exp ops batched per-batch to minimize ACT table reloads.
"""

import sys
from contextlib import ExitStack

if "/opt/trn_rl_repo" not in sys.path:
    sys.path.insert(0, "/opt/trn_rl_repo")

import numpy as np

import concourse.bass as bass
import concourse.bacc as bacc
import concourse.tile as tile
import concourse.mybir as mybir
from concourse import bass_utils
from concourse.masks import make_identity

F32 = mybir.dt.float32
F32R = mybir.dt.float32r
U8 = mybir.dt.uint8
AF = mybir.ActivationFunctionType
ALU = mybir.AluOpType

# Problem dims (hardcoded per spec)
B_FULL, K, S, D, KVH = 32, 8, 2048, 256, 2
NCORES = 8
BL = B_FULL // NCORES          # 4 batches per core
F = KVH * D                    # 512 = k/v projection width
G = K // KVH                   # 4 query heads per kv head
EPS = 1e-6
SCALING = D ** -0.5
NST = 4                        # s-tiles per batch (512 positions each)
STW = S // NST                 # 512
NSC = S // 128                 # 16 s-chunks of 128
MASK_NEG = -1.0e30


def build_kernel(nc, tc, ctx):
    tgt_d = nc.dram_tensor("target", [BL, K, D], F32R, kind="ExternalInput")
    hist_d = nc.dram_tensor("history", [BL, S, D], F32R, kind="ExternalInput")
    mask_d = nc.dram_tensor("mask", [BL, S], U8, kind="ExternalInput")
    wq_d = nc.dram_tensor("wq", [D, D], F32R, kind="ExternalInput")
    wk_d = nc.dram_tensor("wk", [F, D], F32R, kind="ExternalInput")
    wv_d = nc.dram_tensor("wv", [F, D], F32R, kind="ExternalInput")
    gq_d = nc.dram_tensor("gq", [1, D], F32, kind="ExternalInput")
    gk_d = nc.dram_tensor("gk", [1, D], F32, kind="ExternalInput")
    out_d = nc.dram_tensor("out", [BL, K, D], F32, kind="ExternalOutput")
    attn_d = nc.dram_tensor("attnw", [BL, K, S], F32, kind="ExternalOutput")

    consts = ctx.enter_context(tc.tile_pool(name="consts", bufs=1))
    # Single PSUM pool; per-tag bufs sum to exactly 8 banks:
    # ptr:2 + pk:2 + psmall:4
    psum = ctx.enter_context(tc.tile_pool(name="psum", bufs=1, space="PSUM"))
    hpool = ctx.enter_context(tc.tile_pool(name="hpool", bufs=10))
    htpool = ctx.enter_context(tc.tile_pool(name="htpool", bufs=4))
    sqpool = ctx.enter_context(tc.tile_pool(name="sqpool", bufs=6))
    bpool = ctx.enter_context(tc.tile_pool(name="bpool", bufs=3))
    stpool = ctx.enter_context(tc.tile_pool(name="stpool", bufs=3))

    def ptile(tag, bufs, shape=(128, STW), dtype=F32):
        return psum.tile(list(shape), dtype, tag=tag, bufs=bufs, name=tag)

    identf = consts.tile([128, 128], F32)
    make_identity(nc, identf)
    ident = consts.tile([128, 128], F32R)
    nc.vector.tensor_copy(ident, identf)
    epsb = consts.tile([128, 1], F32)
    nc.vector.memset(epsb, EPS)

    # ---- load weights (natural layouts, f32r) ----
    wk_nat = consts.tile([128, 4, D], F32R)    # [f%128, f//128, e]
    nc.sync.dma_start(out=wk_nat, in_=wk_d.rearrange("(c p) e -> p c e", p=128))
    wv_nat = consts.tile([128, 4, D], F32R)
    nc.sync.dma_start(out=wv_nat, in_=wv_d.rearrange("(c p) e -> p c e", p=128))
    wq_nat = consts.tile([128, 2, D], F32R)    # [e%128, e//128, d]
    nc.sync.dma_start(out=wq_nat, in_=wq_d.rearrange("(c p) e -> p c e", p=128))

    # ---- transposed weights: wkT/wvT [e, f] as [128, ec, 512]; wqT [d, e] ----
    wkT = consts.tile([128, 2, F], F32R)
    wvT = consts.tile([128, 2, F], F32R)
    for (w_nat, wT) in ((wk_nat, wkT), (wv_nat, wvT)):
        for ec in range(2):
            ps = ptile("ptr", 2, dtype=F32R)
            for fc in range(4):
                nc.tensor.transpose(
                    ps[:, fc * 128:(fc + 1) * 128],
                    w_nat[:, fc, ec * 128:(ec + 1) * 128],
                    ident,
                )
            nc.vector.tensor_copy(wT[:, ec, :], ps)
    wqT = consts.tile([128, 2, D], F32R)       # [d%128, d//128, e]
    for dc in range(2):
        ps = ptile("ptr", 2, dtype=F32R)
        for ec in range(2):
            nc.tensor.transpose(
                ps[:, ec * 128:(ec + 1) * 128],
                wq_nat[:, ec, dc * 128:(dc + 1) * 128],
                ident,
            )
        nc.vector.tensor_copy(wqT[:, dc, :], ps[:, 0:D])

    # ---- gamma product vector: gprodT[d%128, d//128] = (1+gq)*(1+gk)*SCALING ----
    gqT_t = consts.tile([128, 2], F32)
    gkT_t = consts.tile([128, 2], F32)
    nc.sync.dma_start(out=gqT_t, in_=gq_d.rearrange("a (c p) -> (a p) c", p=128))
    nc.sync.dma_start(out=gkT_t, in_=gk_d.rearrange("a (c p) -> (a p) c", p=128))
    gq1 = consts.tile([128, 2], F32)
    gk1 = consts.tile([128, 2], F32)
    nc.vector.tensor_scalar_add(gq1, gqT_t, 1.0)
    nc.vector.tensor_scalar_add(gk1, gkT_t, 1.0)
    gboth = consts.tile([128, 2], F32)
    nc.vector.tensor_tensor(out=gboth, in0=gq1, in1=gk1, op=ALU.mult)
    gprodT = consts.tile([128, 2], F32)       # [d%128, d//128]
    nc.vector.tensor_scalar_mul(gprodT, gboth, SCALING)

    # ---- ones patterns for the sumsq partition-reduce matmuls ----
    onesPf = consts.tile([128, 2, K], F32)
    nc.vector.memset(onesPf, 0.0)
    nc.vector.memset(onesPf[:, 0, 0:G], 1.0)
    nc.vector.memset(onesPf[:, 1, G:K], 1.0)
    onesP = consts.tile([128, 2, K], F32R)
    nc.vector.tensor_copy(onesP, onesPf)

    # ---- mask, broadcast to 8 partitions: [8, b*st, 512] (uint8) ----
    mask_rep = consts.tile([K, BL * NST, STW], U8)
    mask_ap = mask_d[:, :]
    mask_bcast = bass.AP(
        tensor=mask_ap.tensor,
        offset=mask_ap.offset,
        ap=[[0, K], [STW, BL * NST], [1, STW]],
    )
    nc.gpsimd.dma_start(out=mask_rep, in_=mask_bcast)

    # ================= per-batch pipeline =================
    for b in range(BL):
        # ---- q path ----
        tgt = bpool.tile([K, D], F32R, tag="tgt", name="tgt")
        nc.sync.dma_start(out=tgt, in_=tgt_d[b])
        ps_t = ptile("ptr", 2, dtype=F32R)
        for dc in range(2):
            nc.tensor.transpose(
                ps_t[:, dc * K:(dc + 1) * K],
                tgt[:, dc * 128:(dc + 1) * 128],
                ident[0:K, 0:K],
            )
        tgtT = bpool.tile([128, 2 * K], F32R, tag="tgtT", name="tgtT")
        nc.vector.tensor_copy(tgtT, ps_t[:, 0:2 * K])

        ps_q = ptile("psmall", 4, (K, F))
        for dc in range(2):
            nc.tensor.matmul(
                ps_q[:, 0:D], tgtT[:, dc * K:(dc + 1) * K], wqT[:, dc, :],
                start=(dc == 0), stop=(dc == 1),
            )
        qsq = bpool.tile([K, D], F32, tag="qsq", name="qsq")
        ssq = bpool.tile([K, 1], F32, tag="ssq", name="ssq")
        nc.scalar.activation(qsq, ps_q[:, 0:D], AF.Square, accum_out=ssq)
        lnq = bpool.tile([K, 1], F32, tag="lnq", name="lnq")
        nc.scalar.activation(lnq, ssq, AF.Ln, bias=epsb[0:K], scale=1.0 / D)
        rq = bpool.tile([K, 1], F32, tag="rq", name="rq")
        nc.scalar.activation(rq, lnq, AF.Exp, scale=-0.5)
        qn = bpool.tile([K, D], F32R, tag="qn", name="qn")
        nc.vector.tensor_scalar_mul(qn, ps_q[:, 0:D], rq)

        ps_qn = ptile("ptr", 2, dtype=F32R)
        for dc in range(2):
            nc.tensor.transpose(
                ps_qn[:, dc * K:(dc + 1) * K],
                qn[:, dc * 128:(dc + 1) * 128],
                ident[0:K, 0:K],
            )
        qnT = bpool.tile([128, 2 * K], F32R, tag="qnT", name="qnT")
        nc.vector.tensor_copy(qnT, ps_qn[:, 0:2 * K])

        Qtf = bpool.tile([128, 4, K], F32, tag="Qtf", name="Qtf")
        nc.vector.memset(Qtf, 0.0)
        for c in range(4):
            h, dc = c // 2, c % 2
            nc.vector.tensor_scalar_mul(
                Qtf[:, c, h * G:(h + 1) * G],
                qnT[:, dc * K + h * G: dc * K + (h + 1) * G],
                gprodT[:, dc:dc + 1],
            )
        Qt = bpool.tile([128, 4, K], F32R, tag="Qt", name="Qt")
        nc.vector.tensor_copy(Qt, Qtf)

        ps_pt = ptile("psmall", 4, (K, F))
        for c in range(4):
            nc.tensor.matmul(
                ps_pt[:, 0:D], Qt[:, c, :], wk_nat[:, c, :],
                start=(c == 0), stop=(c == 3),
            )
        PT_sb = bpool.tile([K, D], F32R, tag="PT_sb", name="PT_sb")
        nc.vector.tensor_copy(PT_sb, ps_pt[:, 0:D])
        ps_p = ptile("ptr", 2, dtype=F32R)
        for ec in range(2):
            nc.tensor.transpose(
                ps_p[:, ec * K:(ec + 1) * K],
                PT_sb[:, ec * 128:(ec + 1) * 128],
                ident[0:K, 0:K],
            )
        P_sb = bpool.tile([128, 2 * K], F32R, tag="P_sb", name="P_sb")
        nc.vector.tensor_copy(P_sb, ps_p[:, 0:2 * K])

        h_tiles = []
        score_tiles = []
        for st in range(NST):
            h_nat = hpool.tile([128, 4, D], F32R, tag="h_nat", name="h_nat")
            nc.sync.dma_start(
                out=h_nat,
                in_=hist_d[b].rearrange("(t p) e -> p t e", p=128)[
                    :, st * 4:(st + 1) * 4, :
                ],
            )
            h_tiles.append(h_nat)

            # transpose history tile: hT [e, s] chunks
            hT = htpool.tile([128, 2, STW], F32R, tag="hT", name="hT")
            for ec in range(2):
                ps_h = ptile("ptr", 2, dtype=F32R)
                for j2 in range(4):
                    nc.tensor.transpose(
                        ps_h[:, j2 * 128:(j2 + 1) * 128],
                        h_nat[:, j2, ec * 128:(ec + 1) * 128],
                        ident,
                    )
                nc.vector.tensor_copy(hT[:, ec, :], ps_h)

            # scores_raw [8, 512]
            ps_sc = ptile("psmall", 4, (K, STW))
            nc.tensor.matmul(ps_sc, P_sb[:, 0:K], hT[:, 0, :], start=True, stop=False)
            nc.tensor.matmul(ps_sc, P_sb[:, K:2 * K], hT[:, 1, :], start=False,
                             stop=True)

            # k-projection (kT layout) + squares + sumsq partition-reduce
            ps_ss = ptile("psmall", 4, (K, STW))
            for c in range(4):
                ps_k = ptile("pk", 2)
                nc.tensor.matmul(
                    ps_k, wkT[:, 0, c * 128:(c + 1) * 128], hT[:, 0, :],
                    start=True, stop=False,
                )
                nc.tensor.matmul(
                    ps_k, wkT[:, 1, c * 128:(c + 1) * 128], hT[:, 1, :],
                    start=False, stop=True,
                )
                sq = sqpool.tile([128, STW], F32R, tag="sq", name="sq")
                nc.scalar.activation(sq, ps_k, AF.Square)
                nc.tensor.matmul(
                    ps_ss, onesP[:, c // 2, :], sq, start=(c == 0), stop=(c == 3)
                )

            # r = rsqrt(mean+eps) = exp(-0.5*ln(mean+eps)); single ACT table
            lnm = stpool.tile([K, STW], F32, tag="lnm", name="lnm")
            nc.scalar.activation(lnm, ps_ss, AF.Ln, bias=epsb[0:K], scale=1.0 / D)
            rrec = stpool.tile([K, STW], F32, tag="rrec", name="rrec")
            nc.scalar.activation(rrec, lnm, AF.Exp, scale=-0.5)
            scorest = stpool.tile([K, STW], F32, tag="scorest", name="scorest")
            nc.vector.tensor_mul(scorest, ps_sc, rrec)
            scores = stpool.tile([K, STW], F32, tag="scores", name="scores", bufs=6)
            nc.vector.scalar_tensor_tensor(
                out=scores, in0=mask_rep[:, b * NST + st, :], scalar=MASK_NEG,
                in1=scorest, op0=ALU.mult, op1=ALU.add,
            )
            score_tiles.append(scores)

        # ---- batched exp (one ACT table switch per batch) ----
        exp_tiles = []
        sum_tiles = []
        for st in range(NST):
            exp_st = stpool.tile([K, STW], F32R, tag="exp_st", name="exp_st", bufs=6)
            expsum = stpool.tile([K, 1], F32, tag="expsum", name="expsum", bufs=6)
            nc.scalar.activation(exp_st, score_tiles[st], AF.Exp, accum_out=expsum)
            exp_tiles.append(exp_st)
            sum_tiles.append(expsum)

        # ---- denominator + attn output ----
        dn01 = bpool.tile([K, 1], F32, tag="dn01", name="dn01")
        nc.vector.tensor_add(dn01, sum_tiles[0], sum_tiles[1])
        dn23 = bpool.tile([K, 1], F32, tag="dn23", name="dn23")
        nc.vector.tensor_add(dn23, sum_tiles[2], sum_tiles[3])
        denom = bpool.tile([K, 1], F32, tag="denom", name="denom")
        nc.vector.tensor_add(denom, dn01, dn23)
        recip = bpool.tile([K, 1], F32, tag="recip", name="recip")
        nc.vector.reciprocal(recip, denom)

        for st in range(NST):
            attn_st = stpool.tile([K, STW], F32, tag="attn_st", name="attn_st")
            nc.vector.tensor_scalar_mul(attn_st, exp_tiles[st], recip)
            nc.sync.dma_start(
                out=attn_d[b][:, st * STW:(st + 1) * STW], in_=attn_st
            )

        # ---- context: expT chunks then ctx = exp @ history ----
        ps_et = ptile("ptr", 2, dtype=F32R)
        for sc in range(NSC):
            st, cc = sc // 4, sc % 4
            nc.tensor.transpose(
                ps_et[:, sc * K:(sc + 1) * K],
                exp_tiles[st][:, cc * 128:(cc + 1) * 128],
                ident[0:K, 0:K],
            )
        expT = bpool.tile([128, NSC * K], F32R, tag="expT", name="expT")
        nc.vector.tensor_copy(expT, ps_et[:, 0:NSC * K])

        ps_ctx = ptile("psmall", 4, (K, F))
        for sc in range(NSC):
            nc.tensor.matmul(
                ps_ctx[:, 0:D], expT[:, sc * K:(sc + 1) * K],
                h_tiles[sc // 4][:, sc % 4, :],
                start=(sc == 0), stop=(sc == NSC - 1),
            )
        ctx_sb = bpool.tile([K, D], F32R, tag="ctx_sb", name="ctx_sb")
        nc.vector.tensor_copy(ctx_sb, ps_ctx[:, 0:D])
        ps_ct = ptile("ptr", 2, dtype=F32R)
        for ec in range(2):
            nc.tensor.transpose(
                ps_ct[:, ec * K:(ec + 1) * K],
                ctx_sb[:, ec * 128:(ec + 1) * 128],
                ident[0:K, 0:K],
            )
        ctxT = bpool.tile([128, 2 * K], F32R, tag="ctxT", name="ctxT")
        nc.vector.tensor_copy(ctxT, ps_ct[:, 0:2 * K])

        # ---- out = (ctx @ wv^T) * recip, via block-diagonal ctx_aug ----
        ctx_augf = bpool.tile([128, 4, K], F32, tag="ctx_augf", name="ctx_augf")
        nc.vector.memset(ctx_augf, 0.0)
        for c in range(4):
            h, ec = c // 2, c % 2
            nc.vector.tensor_copy(
                ctx_augf[:, c, h * G:(h + 1) * G],
                ctxT[:, ec * K + h * G: ec * K + (h + 1) * G],
            )
        ctx_aug = bpool.tile([128, 4, K], F32R, tag="ctx_aug", name="ctx_aug")
        nc.vector.tensor_copy(ctx_aug, ctx_augf)
        ps_o = ptile("psmall", 4, (K, F))
        for c in range(4):
            nc.tensor.matmul(
                ps_o[:, 0:D], ctx_aug[:, c, :],
                wvT[:, c % 2, (c // 2) * D:((c // 2) + 1) * D],
                start=(c == 0), stop=(c == 3),
            )
        out_sb = bpool.tile([K, D], F32, tag="out_sb", name="out_sb")
        nc.vector.tensor_scalar_mul(out_sb, ps_o[:, 0:D], recip)
        nc.sync.dma_start(out=out_d[b], in_=out_sb)


def _patch_act_tables():
    # Force all our ACT funcs (Square/Ln/Exp/Copy) onto the one shared table
    # so the table-load pass never alternates tables between them.
    import concourse.hw_specs as hw_specs
    if getattr(bacc, "_act_tables_patched", False):
        return
    orig = hw_specs.get_activation_tables

    def patched(arch):
        t = orig(arch)
        shared = t.get("natural_log_exp_and_others")
        if shared is None:
            return t
        return {
            name: (funcs if name == "natural_log_exp_and_others"
                   else funcs - shared)
            for name, funcs in t.items()
        }

    bacc.get_activation_tables = patched
    bacc._act_tables_patched = True


def build_program():
    _patch_act_tables()
    nc = bacc.Bacc("TRN2", target_bir_lowering=False)
    with tile.TileContext(nc) as tc:
        with ExitStack() as ctx:
            build_kernel(nc, tc, ctx)
    nc.compile()
    return nc


_program = None


def _get_program():
    global _program
    if _program is None:
        _program = build_program()
    return _program


def kernel(**inputs):
    target_emb = np.asarray(inputs["target_emb"], dtype=np.float32)
    history_emb = np.asarray(inputs["history_emb"], dtype=np.float32)
    mask = np.asarray(inputs["key_padding_mask"]).astype(np.uint8)
    wq = np.asarray(inputs["wq"], dtype=np.float32)
    wk = np.asarray(inputs["wk"], dtype=np.float32)
    wv = np.asarray(inputs["wv"], dtype=np.float32)
    gq = np.asarray(inputs["gq"], dtype=np.float32).reshape(1, D)
    gk = np.asarray(inputs["gk"], dtype=np.float32).reshape(1, D)

    nc = _get_program()
    in_maps = []
    for i in range(NCORES):
        sl = slice(i * BL, (i + 1) * BL)
        in_maps.append({
            "target": target_emb[sl],
            "history": history_emb[sl],
            "mask": mask[sl],
            "wq": wq, "wk": wk, "wv": wv, "gq": gq, "gk": gk,
        })
    res = bass_utils.run_bass_kernel_spmd(nc, in_maps, core_ids=list(range(NCORES)))
    out = np.concatenate([res.results[i]["out"] for i in range(NCORES)], axis=0)
    attn = np.concatenate([res.results[i]["attnw"] for i in range(NCORES)], axis=0)
    return out, attn.reshape(B_FULL, K, 1, S)
